# revision 1
# baseline (speedup 1.0000x reference)
"""Trainium2 Bass kernel for nn_CrossGRU (2-layer GRU + cross-attention + MLP + normalize).

Self-contained: hardcodes shapes, shards inputs over 8 NeuronCores (data-parallel
over the N axis), runs one SPMD Bass/Tile kernel with AllGather collectives for
the cross-attention S exchange and the final mean/std, and returns the full
[N] output.

Layout invariant on device: every logical [D=256, rows] feature/gate-major
tensor X is an SBUF/PSUM tile [128, D//128, rows] with
tile[p, c, r] = X[128*c + p, r]. All matmul inputs are fp16; PSUM accumulates
fp32.
"""

import math
import os
from contextlib import ExitStack

import numpy as np

import concourse.bass as bass
import concourse.mybir as mybir
import concourse.tile as tile
from concourse import bacc
from concourse.bass_utils import run_bass_kernel_spmd
from concourse.tile_rust import add_dep_helper

# ---- problem constants (hardcoded) ----
N, T, IN = 2048, 128, 64
T = int(os.environ.get("KERNEL_T", T))  # debug-only override
H = 256
NH, HD = 2, 128
C = 30
NCORES = 8
RC = N // NCORES  # rows per core = 256
P = 128
KIN = IN + 1  # x rows + ones row for fused bias

F16 = mybir.dt.float16
F32 = mybir.dt.float32
F8 = mybir.dt.float8e4
DR = mybir.MatmulPerfMode.DoubleRow
USE_FP8 = os.environ.get("KERNEL_FP8", "0") == "1"
PACE = float(os.environ.get("KERNEL_PACE", "0"))     # cycle ns; 0 = off
PACE_T0 = float(os.environ.get("KERNEL_PACE_T0", "8000"))
PHI_ACT = float(os.environ.get("KERNEL_PHI_ACT", "3300"))
PHI_DVE = float(os.environ.get("KERNEL_PHI_DVE", "4300"))
AF = mybir.ActivationFunctionType
OP = mybir.AluOpType

_CACHE = {}


# --------------------------------------------------------------------------
# device kernel construction
# --------------------------------------------------------------------------

def _dep(later, earlier):
    # PE executes matmuls in scheduled order; this edge keeps two PSUM
    # accumulation groups that share a bank from interleaving (a group's
    # start=True clears has_written bits for the whole bank).
    add_dep_helper(later.ins, earlier.ins, reason="psum bank group order")


def _build(trace_label=""):
    nc = bacc.Bacc(
        "TRN2", target_bir_lowering=False, debug=False, num_devices=NCORES
    )

    def din(name, shape, dt):
        return nc.dram_tensor(name, shape, dt, kind="ExternalInput").ap()

    # per-core input
    xe_d = din("xe", [T, KIN, RC], F16)
    # GRU weights (replicated)
    w0_d = din("w0", [KIN, 6, P], F16)        # Wih0^T + fused bias row
    wh0_d = din("wh0", [P, 2, 6, P], F16)     # Whh0^T chunks [p, kc, g, m]
    w1_d = din("w1", [P, 2, 6, P], F16)       # Wih1^T chunks
    wh1_d = din("wh1", [P, 2, 6, P], F16)
    # fp8 copies of the r/z gate chunks of Whh (DoubleRow matmuls)
    wh0_8d = din("wh0_8", [P, 2, 4, P], F8)
    wh1_8d = din("wh1_8", [P, 2, 4, P], F8)
    brz1_d = din("brz1", [P, 4], F32)
    bihn1_d = din("bihn1", [P, 2], F32)
    bhhn0_d = din("bhhn0", [P, 2], F32)
    bhhn1_d = din("bhhn1", [P, 2], F32)
    bhhn0r_d = din("bhhn0r", [1, 2, P], F16)  # bias rows for PSUM ones-mms
    bhhn1r_d = din("bhhn1r", [1, 2, P], F16)
    ones_rc_d = din("ones_rc", [1, RC], F16)
    # attention / head weights
    wq_d = din("wq", [P, 2, NH, HD], F16)
    wk_d = din("wk", [P, 2, NH, HD], F16)
    wv_d = din("wv", [P, 2, NH, HD], F16)
    bq_d = din("bq", [P, NH], F32)
    bk_d = din("bk", [P, NH], F32)
    bv_d = din("bv", [P, NH], F32)
    bv16_d = din("bv16", [1, NH, HD], F16)    # bv as rows for bias-matmul
    rt_d = din("rt", [P, 2, C], F16)          # R^T chunks
    wg_d = din("wg", [P, 2, 1], F16)          # W_gate^T chunks
    bg_d = din("bg", [1, 1], F32)
    w1t_d = din("w1t", [P, 2, 2, P], F16)     # W1^T chunks [p, kc, jc, m]
    b1_d = din("b1", [P, 2], F32)
    w2_d = din("w2", [P, 2, 1], F16)
    b2_d = din("b2", [1, 1], F32)
    ident_d = din("ident", [P, P], F16)
    ones1_d = din("ones1", [1, P], F16)

    y_out_d = nc.dram_tensor("y_out", [1, N], F32, kind="ExternalOutput").ap()
    s_dbg_d = nc.dram_tensor("s_dbg", [P, 2, RC], F32, kind="ExternalOutput").ap()
    b_dbg_d = nc.dram_tensor("b_dbg", [P, NH, C], F32, kind="ExternalOutput").ap()
    yraw_dbg_d = nc.dram_tensor("yraw_dbg", [1, N], F32, kind="ExternalOutput").ap()

    with tile.TileContext(nc) as tc:
        _emit(tc, locals())

    nc.compile()
    return nc


def _emit(tc, d):
    nc = tc.nc
    ctx = ExitStack()
    with ctx:
        consts = ctx.enter_context(tc.tile_pool(name="consts", bufs=1))
        hpool = ctx.enter_context(tc.tile_pool(name="hpool", bufs=3))
        gates = ctx.enter_context(tc.tile_pool(name="gates", bufs=3))
        xep = ctx.enter_context(tc.tile_pool(name="xep", bufs=4))

        def cl(name, dram_ap, shape, dt):
            t = consts.tile(shape, dt, name=name)
            nc.sync.dma_start(t[:], dram_ap[:])
            return t

        w0_sb = cl("w0_sb", d["w0_d"], [KIN, 6, P], F16)
        wh0_sb = cl("wh0_sb", d["wh0_d"], [P, 2, 6, P], F16)
        w1_sb = cl("w1_sb", d["w1_d"], [P, 2, 6, P], F16)
        wh1_sb = cl("wh1_sb", d["wh1_d"], [P, 2, 6, P], F16)
        wh0_8sb = cl("wh0_8sb", d["wh0_8d"], [P, 2, 4, P], F8)
        wh1_8sb = cl("wh1_8sb", d["wh1_8d"], [P, 2, 4, P], F8)
        brz1_sb = cl("brz1_sb", d["brz1_d"], [P, 4], F32)
        bihn1_sb = cl("bihn1_sb", d["bihn1_d"], [P, 2], F32)
        bhhn0_sb = cl("bhhn0_sb", d["bhhn0_d"], [P, 2], F32)
        bhhn1_sb = cl("bhhn1_sb", d["bhhn1_d"], [P, 2], F32)
        wq_sb = cl("wq_sb", d["wq_d"], [P, 2, NH, HD], F16)
        wk_sb = cl("wk_sb", d["wk_d"], [P, 2, NH, HD], F16)
        wv_sb = cl("wv_sb", d["wv_d"], [P, 2, NH, HD], F16)
        bq_sb = cl("bq_sb", d["bq_d"], [P, NH], F32)
        bk_sb = cl("bk_sb", d["bk_d"], [P, NH], F32)
        bv_sb = cl("bv_sb", d["bv_d"], [P, NH], F32)
        bv16_sb = cl("bv16_sb", d["bv16_d"], [1, NH, HD], F16)
        rt_sb = cl("rt_sb", d["rt_d"], [P, 2, C], F16)
        wg_sb = cl("wg_sb", d["wg_d"], [P, 2, 1], F16)
        bg_sb = cl("bg_sb", d["bg_d"], [1, 1], F32)
        w1t_sb = cl("w1t_sb", d["w1t_d"], [P, 2, 2, P], F16)
        b1_sb = cl("b1_sb", d["b1_d"], [P, 2], F32)
        w2_sb = cl("w2_sb", d["w2_d"], [P, 2, 1], F16)
        b2_sb = cl("b2_sb", d["b2_d"], [1, 1], F32)
        ident_sb = cl("ident_sb", d["ident_d"], [P, P], F16)
        ones1_sb = cl("ones1_sb", d["ones1_d"], [1, P], F16)

        mm = nc.tensor.matmul
        act = nc.scalar.activation
        V = nc.vector

        # ------------------------------------------------------------------
        # GRU phase
        # ------------------------------------------------------------------
        h0_prev = hpool.tile([P, 2, RC], F16, tag="h0")
        V.memset(h0_prev[:], 0.0)
        h1_prev = hpool.tile([P, 2, RC], F16, tag="h1")
        V.memset(h1_prev[:], 0.0)
        h0_8prev = hpool.tile([P, 2, RC], F8, tag="h0_8")
        V.memset(h0_8prev[:], 0.0)
        h1_8prev = hpool.tile([P, 2, RC], F8, tag="h1_8")
        V.memset(h1_8prev[:], 0.0)

        with tc.tile_pool(name="gru_ps", bufs=1, space="PSUM") as ps:

            def l0_step(t, h_prev, h8_prev):
                xe_t = xep.tile([KIN, RC], F16, tag="xe")
                nc.sync.dma_start(xe_t[:], d["xe_d"][t])

                r_ps = ps.tile([P, 2, RC], F32, tag="r0")
                z_ps = ps.tile([P, 2, RC], F32, tag="z0")
                xn_ps = ps.tile([P, 2, RC], F32, tag="xn0")
                ghn_ps = ps.tile([P, 2, RC], F32, tag="ghn0")

                # r bank (gates 0..255), then ghn, xn, z — r first for the
                # chain. r/z hidden paths run as one fp8 DoubleRow matmul.
                for ps_t, g0, with_in, with_hh, brow in (
                    (r_ps, 0, True, True, None),
                    (ghn_ps, 4, False, True, None),
                    (xn_ps, 4, True, False, None),
                    (z_ps, 2, True, True, None),
                ):
                    prev_last = None
                    for c in (0, 1):
                        g = g0 + c
                        first = True
                        mfirst = None
                        mlast = None
                        if with_in:
                            mlast = mm(ps_t[:, c, :], w0_sb[:, g, :], xe_t[:],
                                       start=True, stop=not with_hh)
                            mfirst = mlast
                            first = False
                        if with_hh:
                            if USE_FP8 and g0 < 4:  # r/z: fp8 DoubleRow
                                mlast = mm(ps_t[:, c, :], wh0_8sb[:, 0:2, g, :],
                                           h8_prev[:, 0:2, :], perf_mode=DR,
                                           start=first, stop=True)
                                if mfirst is None:
                                    mfirst = mlast
                            else:
                                for kc in (0, 1):
                                    mlast = mm(ps_t[:, c, :],
                                               wh0_sb[:, kc, g, :],
                                               h_prev[:, kc, :], start=first,
                                               stop=(kc == 1
                                                     and brow is None))
                                    if mfirst is None:
                                        mfirst = mlast
                                    first = False
                        if brow is not None:
                            mlast = mm(ps_t[:, c, :], brow[:, c, :],
                                       ones_rc_sb[:], start=False, stop=True)
                        if prev_last is not None and (with_in + 2 * with_hh) > 1:
                            _dep(mfirst, prev_last)
                        prev_last = mlast

                r_sb = gates.tile([P, 2, RC], F16, tag="r0s")
                act(r_sb[:], r_ps[:], AF.Sigmoid)
                z_sb = gates.tile([P, 2, RC], F16, tag="z0s")
                act(z_sb[:], z_ps[:], AF.Sigmoid)

                # off-chain: zc = 1-z (Pool), p = z*h (DVE)
                zc_sb = gates.tile([P, 2, RC], F16, tag="zc0s")
                nc.gpsimd.tensor_scalar(zc_sb[:], z_sb[:], -1.0, 1.0,
                                        op0=OP.mult, op1=OP.add)
                p_sb = gates.tile([P, 2, RC], F16, tag="p0s")
                pi = V.tensor_mul(p_sb[:], z_sb[:], h_prev[:])

                with tc.high_priority():
                    t_sb = gates.tile([P, 2, RC], F16, tag="t0s")
                    for c in (0, 1):
                        V.scalar_tensor_tensor(
                            t_sb[:, c, :], ghn_ps[:, c, :],
                            bhhn0_sb[:, c:c + 1],
                            r_sb[:, c, :], op0=OP.add, op1=OP.mult)
                    u_sb = gates.tile([P, 2, RC], F16, tag="u0s")
                    ui = V.tensor_add(u_sb[:], t_sb[:], xn_ps[:])
                    n_sb = gates.tile([P, 2, RC], F16, tag="n0s")
                    ni = act(n_sb[:], u_sb[:], AF.Tanh)

                    # chain tail: h = n*(1-z) + z*h
                    q_sb = gates.tile([P, 2, RC], F16, tag="q0s")
                    V.tensor_mul(q_sb[:], n_sb[:], zc_sb[:])
                    if os.environ.get("KERNEL_E_C", "1") == "1":
                        _dep(pi, ui)  # keep p out of the chain window
                    h_new = hpool.tile([P, 2, RC], F16, tag="h0")
                    hi = V.tensor_add(h_new[:], q_sb[:], p_sb[:])
                if USE_FP8:
                    h8_new = hpool.tile([P, 2, RC], F8, tag="h0_8")
                    nc.gpsimd.tensor_copy(h8_new[:], h_new[:])
                else:
                    h8_new = h8_prev
                return h_new, h8_new, hi, ni

            def l1_step(t, h_prev, h8_prev, h0_t, h0i, n0i):
                r_ps = ps.tile([P, 2, RC], F32, tag="r1")
                z_ps = ps.tile([P, 2, RC], F32, tag="z1")
                xn_ps = ps.tile([P, 2, RC], F32, tag="xn1")
                ghn_ps = ps.tile([P, 2, RC], F32, tag="ghn1")

                for ps_t, g0, with_in, with_hh, brow in (
                    (r_ps, 0, True, True, None),
                    (ghn_ps, 4, False, True, None),
                    (xn_ps, 4, True, False, None),
                    (z_ps, 2, True, True, None),
                ):
                    prev_last = None
                    for c in (0, 1):
                        g = g0 + c
                        mfirst = None
                        mlast = None
                        first = True
                        # hidden part first (h1_prev is ready before h0_t)
                        if with_hh:
                            if USE_FP8 and g0 < 4:  # r/z: fp8 DoubleRow
                                mlast = mm(ps_t[:, c, :], wh1_8sb[:, 0:2, g, :],
                                           h8_prev[:, 0:2, :], perf_mode=DR,
                                           start=True, stop=not with_in)
                                mfirst = mlast
                                first = False
                            else:
                                for kc in (0, 1):
                                    mlast = mm(ps_t[:, c, :],
                                               wh1_sb[:, kc, g, :],
                                               h_prev[:, kc, :], start=first,
                                               stop=(not with_in and kc == 1
                                                     and brow is None))
                                    if mfirst is None:
                                        mfirst = mlast
                                    first = False
                        if with_in:
                            for kc in (0, 1):
                                mlast = mm(ps_t[:, c, :], w1_sb[:, kc, g, :],
                                           h0_t[:, kc, :], start=first,
                                           stop=(kc == 1))
                                if mfirst is None:
                                    mfirst = mlast
                                first = False
                        if brow is not None:
                            mlast = mm(ps_t[:, c, :], brow[:, c, :],
                                       ones_rc_sb[:], start=False, stop=True)
                        if prev_last is not None:
                            _dep(mfirst, prev_last)
                        prev_last = mlast

                r_sb = gates.tile([P, 2, RC], F16, tag="r1s")
                z_sb = gates.tile([P, 2, RC], F16, tag="z1s")
                first_act = None
                for c in (0, 1):
                    a = act(r_sb[:, c, :], r_ps[:, c, :], AF.Sigmoid,
                            bias=brz1_sb[:, c:c + 1])
                    if first_act is None:
                        first_act = a
                for c in (0, 1):
                    act(z_sb[:, c, :], z_ps[:, c, :], AF.Sigmoid,
                        bias=brz1_sb[:, 2 + c:3 + c])
                if n0i is not None and os.environ.get("KERNEL_E_A") == "1":
                    _dep(first_act, n0i)  # ACT: L0 tanh before L1 sigmas

                # off-chain: zc = 1-z (Pool), p = z*h (DVE)
                zc_sb = gates.tile([P, 2, RC], F16, tag="zc1s")
                nc.gpsimd.tensor_scalar(zc_sb[:], z_sb[:], -1.0, 1.0,
                                        op0=OP.mult, op1=OP.add)
                p_sb = gates.tile([P, 2, RC], F16, tag="p1s")
                pi = V.tensor_mul(p_sb[:], z_sb[:], h_prev[:])

                with tc.high_priority():
                    t_sb = gates.tile([P, 2, RC], F16, tag="t1s")
                    first_stt = None
                    for c in (0, 1):
                        a = V.scalar_tensor_tensor(
                            t_sb[:, c, :], ghn_ps[:, c, :],
                            bhhn1_sb[:, c:c + 1],
                            r_sb[:, c, :], op0=OP.add, op1=OP.mult)
                        if first_stt is None:
                            first_stt = a
                    if h0i is not None and os.environ.get("KERNEL_E_B") == "1":
                        _dep(first_stt, h0i)  # DVE: L0 chain before L1 block
                    u_sb = gates.tile([P, 2, RC], F16, tag="u1s")
                    ui = V.tensor_add(u_sb[:], t_sb[:], xn_ps[:])
                    if h0i is not None and \
                            os.environ.get("KERNEL_E_D", "0") == "1":
                        _dep(ui, h0i)  # DVE: L0 h' before L1 u
                    n_sb = gates.tile([P, 2, RC], F16, tag="n1s")
                    for c in (0, 1):
                        act(n_sb[:, c, :], u_sb[:, c, :], AF.Tanh,
                            bias=bihn1_sb[:, c:c + 1])

                    # chain tail: h = n*(1-z) + z*h
                    if os.environ.get("KERNEL_E_C", "1") == "1":
                        _dep(pi, ui)
                    q_sb = gates.tile([P, 2, RC], F16, tag="q1s")
                    V.tensor_mul(q_sb[:], n_sb[:], zc_sb[:])
                    h_new = hpool.tile([P, 2, RC], F16, tag="h1")
                    V.tensor_add(h_new[:], q_sb[:], p_sb[:])
                if USE_FP8:
                    h8_new = hpool.tile([P, 2, RC], F8, tag="h1_8")
                    nc.gpsimd.tensor_copy(h8_new[:], h_new[:])
                else:
                    h8_new = h8_prev
                return h_new, h8_new

            # software-pipelined: L1 lags L0 by one step so PE never stalls
            LAG = int(os.environ.get("KERNEL_LAG", "1"))
            h0_cur, h0_8cur = h0_prev, h0_8prev
            h1_cur, h1_8cur = h1_prev, h1_8prev
            h0_at = {}
            for t in range(T):
                h0_cur, h0_8cur, h0i, n0i = l0_step(t, h0_cur, h0_8cur)
                h0_at[t] = h0_cur
                if t >= LAG:
                    h1_cur, h1_8cur = l1_step(t - LAG, h1_cur, h1_8cur,
                                              h0_at[t - LAG], h0i, n0i)
                    del h0_at[t - LAG]
            for t in range(T - LAG, T):
                h1_cur, h1_8cur = l1_step(t, h1_cur, h1_8cur, h0_at[t],
                                          None, None)
            S = h1_cur



        nc.gpsimd.dma_start(d["s_dbg_d"][:], S[:])  # fp16 -> fp32 cast dma

        # ------------------------------------------------------------------
        # attention + MLP phase (replicated small work + own rows)
        # ------------------------------------------------------------------
        att = ctx.enter_context(tc.tile_pool(name="att", bufs=1))
        rsqd = 1.0 / math.sqrt(HD)

        # alpha = sigmoid(S @ Wg + bg)  (sigmoid table still loaded)
        with tc.tile_pool(name="aps1", bufs=1, space="PSUM") as ap1:
            al_ps = ap1.tile([1, RC], F32, tag="al")
            m1 = mm(al_ps[:], wg_sb[:, 0, :], S[:, 0, :], start=True, stop=False)
            mm(al_ps[:], wg_sb[:, 1, :], S[:, 1, :], start=False, stop=True)
            al16 = att.tile([1, RC], F16)
            act(al16[:], al_ps[:], AF.Sigmoid, bias=bg_sb[:, 0:1])
            alb_ps = ap1.tile([P, RC], F32, tag="alb")
            mm(alb_ps[:], ones1_sb[:], al16[:], start=True, stop=True)
            alb_sb = att.tile([P, RC], F16)
            V.tensor_copy(alb_sb[:], alb_ps[:])

            # qST (own rows, sigmoid-set Identity bias-add)
            qst_sb = att.tile([P, NH, RC], F16)
            for h in (0, 1):
                q_ps = ap1.tile([P, RC], F32, tag="qst")
                mm(q_ps[:], wq_sb[:, 0, h, :], S[:, 0, :], start=True, stop=False)
                mm(q_ps[:], wq_sb[:, 1, h, :], S[:, 1, :], start=False, stop=True)
                nc.scalar.add(qst_sb[:, h, :], q_ps[:], bq_sb[:, h:h + 1])

        # AllGather S across the 8 cores
        sg_in, _f1 = tc.tile([P, 2, RC], F16, space="DRAM", name="sg_in")
        sg_out, _f2 = tc.tile([NCORES, P, 2, RC], F16, space="DRAM",
                              addr_space="Shared", name="sg_out")
        nc.sync.dma_start(sg_in[:], S[:])
        if os.environ.get("KERNEL_NO_CC") == "1":  # timing-sim only
            nc.sync.dma_start(sg_out[0], sg_in[:])
        else:
            nc.gpsimd.collective_compute(
                "AllGather", OP.bypass,
                replica_groups=[list(range(NCORES))],
                ins=[sg_in[:].opt()], outs=[sg_out[:].opt()])
        sfull4 = att.tile([P, 2, NCORES, RC], F16)
        for k in range(NCORES):
            nc.sync.dma_start(sfull4[:, :, k, :], sg_out[k])
        # flat [P, 2, N] view of the same tile (free layout is contiguous)
        sfull = sfull4[:].rearrange("p c k r -> p c (k r)")

        # K^T and V-row projections of the gathered S
        khT = att.tile([P, NH, N], F16)
        vrow = att.tile([P, NH, 16, HD], F16)
        with tc.tile_pool(name="aps2", bufs=2, space="PSUM") as ap2:
            for h in (0, 1):
                for i in range(4):
                    sl = slice(512 * i, 512 * (i + 1))
                    k_ps = ap2.tile([P, 512], F32, tag="kh")
                    mm(k_ps[:], wk_sb[:, 0, h, :], sfull[:, 0, sl],
                       start=True, stop=False)
                    mm(k_ps[:], wk_sb[:, 1, h, :], sfull[:, 1, sl],
                       start=False, stop=True)
                    nc.scalar.add(khT[:, h, sl], k_ps[:], bk_sb[:, h:h + 1])
            for h in (0, 1):
                for a in range(4):  # 4 row-chunks per psum tile
                    v_ps = ap2.tile([P, 4, HD], F32, tag="vr")
                    prev_last = None
                    for j in range(4):
                        rc = 4 * a + j
                        sl = slice(P * rc, P * (rc + 1))
                        ma = mm(v_ps[:, j, :], sfull[:, 0, sl], wv_sb[:, 0, h, :],
                                start=True, stop=False)
                        mm(v_ps[:, j, :], sfull[:, 1, sl], wv_sb[:, 1, h, :],
                           start=False, stop=False)
                        mb = mm(v_ps[:, j, :], ones1_sb[:], bv16_sb[:, h, :],
                                start=False, stop=True)
                        if prev_last is not None:
                            _dep(ma, prev_last)
                        prev_last = mb
                    # alternate drain engine so DVE and ACT overlap
                    if a % 2 == 0:
                        V.tensor_copy(vrow[:, h, 4 * a:4 * a + 4, :], v_ps[:])
                    else:
                        nc.scalar.copy(vrow[:, h, 4 * a:4 * a + 4, :], v_ps[:])

            # q_R^T
            qrt = att.tile([P, NH, C], F16)
            for h in (0, 1):
                q_ps = ap2.tile([P, C], F32, tag="qr")
                mm(q_ps[:], wq_sb[:, 0, h, :], rt_sb[:, 0, :], start=True,
                   stop=False)
                mm(q_ps[:], wq_sb[:, 1, h, :], rt_sb[:, 1, :], start=False,
                   stop=True)
                nc.scalar.add(qrt[:, h, :], q_ps[:], bq_sb[:, h:h + 1])


        # scores for B = mha(R, S, S): [C, N] per head, softmax along free
        expT = att.tile([P, NH, 16, C], F16)
        sums_b = att.tile([C, NH], F32)
        with tc.tile_pool(name="aps3", bufs=1, space="PSUM") as ap3:
            rec_bcs = {}
            for h in (0, 1):
                e_sb = att.tile([C, N], F16, tag="eB", bufs=2)
                psums = att.tile([C, 2], F32, tag="psumsB", bufs=2)
                for half in (0, 1):
                    s_ps = ap3.tile([C, N // 2], F32, tag=f"s{half}")
                    for i in range(2):
                        so = slice(512 * i, 512 * (i + 1))
                        si = slice(1024 * half + 512 * i,
                                   1024 * half + 512 * (i + 1))
                        mm(s_ps[:, so], qrt[:, h, :], khT[:, h, si],
                           start=True, stop=True)
                    act(e_sb[:, 1024 * half:1024 * (half + 1)], s_ps[:],
                        AF.Exp, scale=rsqd,
                        accum_out=psums[:, half:half + 1])
                V.tensor_add(sums_b[:, h:h + 1], psums[:, 0:1], psums[:, 1:2])
                rec = att.tile([C, 1], F32, tag="recB", bufs=2)
                V.reciprocal(rec[:], sums_b[:, h:h + 1])
                # 1/sum broadcast to [P, C]; the softmax normalization is
                # applied to B^T after the matmul (diag scale commutes).
                rec16 = att.tile([C, 1], F16, tag="rec16", bufs=2)
                V.tensor_copy(rec16[:], rec[:])
                recT_ps = ap3.tile([1, C], F16, tag="recT", bufs=1)
                nc.tensor.transpose(recT_ps[:], rec16[:], ident_sb[:C, :C])
                recT_sb = att.tile([1, C], F16, tag="recTs", bufs=2)
                V.tensor_copy(recT_sb[:], recT_ps[:])
                rec_bc_ps = ap3.tile([P, C], F32, tag="recbc", bufs=1)
                mm(rec_bc_ps[:], ones1_sb[:], recT_sb[:], start=True,
                   stop=True)
                rec_bc = att.tile([P, C], F16, tag="recbcs", bufs=2)
                V.tensor_copy(rec_bc[:], rec_bc_ps[:])
                rec_bcs[h] = rec_bc
                # transpose unnormalized exp into [rows, C] chunks
                for a in range(2):
                    tr_ps = ap3.tile([P, 8, C], F16, tag="tr", bufs=1)
                    for j in range(8):
                        rc = 8 * a + j
                        nc.tensor.transpose(
                            tr_ps[:, j, :], e_sb[:, P * rc:P * (rc + 1)],
                            ident_sb[:C, :C])
                    if a == 0:
                        V.tensor_copy(expT[:, h, 8 * a:8 * a + 8, :], tr_ps[:])
                    else:
                        nc.scalar.copy(expT[:, h, 8 * a:8 * a + 8, :],
                                       tr_ps[:])

            # B^T per head: accumulate vrow^T @ expT over 16 row chunks
            bt_sb = att.tile([P, NH, C], F16)
            for h in (0, 1):
                bt_ps = ap3.tile([P, C], F32, tag="bt")
                for rc in range(16):
                    mm(bt_ps[:], vrow[:, h, rc, :], expT[:, h, rc, :],
                       start=(rc == 0), stop=(rc == 15))
                V.tensor_mul(bt_sb[:, h, :], bt_ps[:], rec_bcs[h][:])



        nc.gpsimd.dma_start(d["b_dbg_d"][:], bt_sb[:])

        # second attention: S' = mha(S, B, B) for own rows
        with tc.tile_pool(name="aps4", bufs=1, space="PSUM") as ap4:
            kb_sb = att.tile([P, NH, C], F16)
            vb2_sb = att.tile([P, NH, C], F16)
            for h in (0, 1):
                kb_ps = ap4.tile([P, C], F32, tag="kb")
                mm(kb_ps[:], wk_sb[:, 0, h, :], bt_sb[:, 0, :], start=True,
                   stop=False)
                mm(kb_ps[:], wk_sb[:, 1, h, :], bt_sb[:, 1, :], start=False,
                   stop=True)
                nc.scalar.add(kb_sb[:, h, :], kb_ps[:], bk_sb[:, h:h + 1])
                vb_ps = ap4.tile([P, C], F32, tag="vb")
                mm(vb_ps[:], wv_sb[:, 0, h, :], bt_sb[:, 0, :], start=True,
                   stop=False)
                mm(vb_ps[:], wv_sb[:, 1, h, :], bt_sb[:, 1, :], start=False,
                   stop=True)
                nc.scalar.add(vb2_sb[:, h, :], vb_ps[:], bv_sb[:, h:h + 1])
            # scores2 + softmax + S'
            sp_sb = att.tile([P, 2, RC], F16)
            p2 = att.tile([P, NH, 2, C], F16)
            for h in (0, 1):
                for rc2 in (0, 1):
                    s2_ps = ap4.tile([P, C], F32, tag="s2")
                    mm(s2_ps[:], qst_sb[:, h, P * rc2:P * (rc2 + 1)],
                       kb_sb[:, h, :], start=True, stop=True)
                    e2 = att.tile([P, C], F16, tag="e2", bufs=4)
                    ssum = att.tile([P, 1], F32, tag="ssum", bufs=4)
                    act(e2[:], s2_ps[:], AF.Exp, scale=rsqd, accum_out=ssum[:])
                    rec2 = att.tile([P, 1], F32, tag="rec2", bufs=4)
                    V.reciprocal(rec2[:], ssum[:])
                    V.tensor_scalar_mul(p2[:, h, rc2, :], e2[:], rec2[:])
            # transpose p2 and vb to put C on partitions
            p2t = att.tile([C, NH, 2, P], F16)
            vbr = att.tile([C, NH, P], F16)
            for h in (0, 1):
                t_ps = ap4.tile([C, 2, P], F16, tag="p2t")
                for rc2 in (0, 1):
                    nc.tensor.transpose(t_ps[:, rc2, :], p2[:, h, rc2, :],
                                        ident_sb[:, :])
                V.tensor_copy(p2t[:, h, :, :], t_ps[:])
                v_ps = ap4.tile([C, P], F16, tag="vbt")
                nc.tensor.transpose(v_ps[:], vb2_sb[:, h, :], ident_sb[:, :])
                V.tensor_copy(vbr[:, h, :], v_ps[:])
            for h in (0, 1):
                o_ps = ap4.tile([P, RC], F32, tag="o2")
                for rc2 in (0, 1):
                    mm(o_ps[:, P * rc2:P * (rc2 + 1)], vbr[:, h, :],
                       p2t[:, h, rc2, :], start=True, stop=True)
                V.tensor_copy(sp_sb[:, h, :], o_ps[:])

            # S_mix = S + alpha * (S' - S)
            dd = att.tile([P, 2, RC], F16)
            V.tensor_sub(dd[:], sp_sb[:], S[:])
            ee = att.tile([P, 2, RC], F16)
            for c in (0, 1):
                V.tensor_mul(ee[:, c, :], alb_sb[:], dd[:, c, :])
            mix = att.tile([P, 2, RC], F16)
            V.tensor_add(mix[:], S[:], ee[:])

            # MLP
            v1_sb = att.tile([P, 2, RC], F16)
            for jc in (0, 1):
                v1_ps = ap4.tile([P, RC], F32, tag="v1")
                mm(v1_ps[:], w1t_sb[:, 0, jc, :], mix[:, 0, :],
                   start=True, stop=False)
                mm(v1_ps[:], w1t_sb[:, 1, jc, :], mix[:, 1, :],
                   start=False, stop=True)
                act(v1_sb[:, jc, :], v1_ps[:], AF.Relu, bias=b1_sb[:, jc:jc + 1])
            s2m = att.tile([P, 2, RC], F16)
            V.tensor_add(s2m[:], v1_sb[:], mix[:])
            y_ps = ap4.tile([1, RC], F32, tag="y")
            mm(y_ps[:], w2_sb[:, 0, :], s2m[:, 0, :], start=True, stop=False)
            mm(y_ps[:], w2_sb[:, 1, :], s2m[:, 1, :], start=False, stop=True)
            y_sb = att.tile([1, RC], F32)
            nc.scalar.add(y_sb[:], y_ps[:], b2_sb[:, 0:1])

        # gather y across cores, compute mean/std, normalize
        yg_in, _f3 = tc.tile([1, RC], F32, space="DRAM", name="yg_in")
        yg_out, _f4 = tc.tile([NCORES, RC], F32, space="DRAM",
                              addr_space="Shared", name="yg_out")
        nc.sync.dma_start(yg_in[:], y_sb[:])
        if os.environ.get("KERNEL_NO_CC") == "1":  # timing-sim only
            nc.sync.dma_start(yg_out[0], yg_in[:])
        else:
            nc.gpsimd.collective_compute(
                "AllGather", OP.bypass,
                replica_groups=[list(range(NCORES))],
                ins=[yg_in[:].opt()], outs=[yg_out[:].opt()])
        yf = att.tile([1, N], F32)
        nc.sync.dma_start(yf[:], yg_out[:].rearrange("k r -> (k r)"))

        nc.sync.dma_start(d["yraw_dbg_d"][:], yf[:])
        dumm = att.tile([1, N], F32)
        tsum = att.tile([1, 1], F32)
        act(dumm[:], yf[:], AF.Identity, accum_out=tsum[:])
        dumm2 = att.tile([1, N], F32)
        tsq = att.tile([1, 1], F32)
        act(dumm2[:], yf[:], AF.Square, accum_out=tsq[:])
        mean = att.tile([1, 1], F32)
        V.tensor_scalar_mul(mean[:], tsum[:], 1.0 / N)
        m2 = att.tile([1, 1], F32)
        V.tensor_mul(m2[:], mean[:], mean[:])
        nm2 = att.tile([1, 1], F32)
        V.tensor_scalar_mul(nm2[:], m2[:], -float(N))
        varn = att.tile([1, 1], F32)
        V.tensor_add(varn[:], tsq[:], nm2[:])
        var = att.tile([1, 1], F32)
        V.tensor_scalar_mul(var[:], varn[:], 1.0 / (N - 1))
        std = att.tile([1, 1], F32)
        act(std[:], var[:], AF.Sqrt)
        stdp = att.tile([1, 1], F32)
        V.tensor_scalar_add(stdp[:], std[:], 1e-8)
        inv = att.tile([1, 1], F32)
        V.reciprocal(inv[:], stdp[:])
        yo = att.tile([1, N], F32)
        V.tensor_scalar(yo[:], yf[:], mean[:], inv[:],
                        op0=OP.subtract, op1=OP.mult)
        nc.sync.dma_start(d["y_out_d"][:], yo[:])

        _f1(), _f2(), _f3(), _f4()


# --------------------------------------------------------------------------
# host side
# --------------------------------------------------------------------------

def _prep_host(inputs):
    f16 = np.float16
    f32 = np.float32
    d = {k: np.asarray(v) for k, v in inputs.items()}

    shared = {}
    W_ih0 = d["W_ih0"].astype(f32)
    b_ih0 = d["b_ih0"].astype(f32)
    b_hh0 = d["b_hh0"].astype(f32)
    w0 = np.zeros((KIN, 3 * H), f32)
    w0[:IN] = W_ih0.T
    w0[IN, :2 * H] = (b_ih0 + b_hh0)[:2 * H]
    w0[IN, 2 * H:] = b_ih0[2 * H:]
    shared["w0"] = np.ascontiguousarray(w0.reshape(KIN, 6, P).astype(f16))

    def whh_layout(W):  # [3H, H] -> [P, 2, 6, P]
        Wt = W.T.astype(f32)  # [H, 3H]
        return np.ascontiguousarray(
            Wt.reshape(2, P, 6, P).transpose(1, 0, 2, 3).astype(f16))

    shared["wh0"] = whh_layout(d["W_hh0"])
    shared["w1"] = whh_layout(d["W_ih1"])
    shared["wh1"] = whh_layout(d["W_hh1"])

    import ml_dtypes
    f8 = ml_dtypes.float8_e4m3

    def whh8_rz(W):  # r/z gate chunks of Whh^T as fp8 [P, 2, 4, P]
        Wt = W.T.astype(f32)
        full = Wt.reshape(2, P, 6, P).transpose(1, 0, 2, 3)
        return np.ascontiguousarray(full[:, :, 0:4, :]).astype(f8)

    shared["wh0_8"] = whh8_rz(d["W_hh0"])
    shared["wh1_8"] = whh8_rz(d["W_hh1"])
    b_ih1 = d["b_ih1"].astype(f32)
    b_hh1 = d["b_hh1"].astype(f32)
    shared["brz1"] = np.ascontiguousarray(
        (b_ih1 + b_hh1)[:2 * H].reshape(4, P).T.astype(f32))
    shared["bihn1"] = np.ascontiguousarray(
        b_ih1[2 * H:].reshape(2, P).T.astype(f32))
    shared["bhhn0"] = np.ascontiguousarray(
        b_hh0[2 * H:].reshape(2, P).T.astype(f32))
    shared["bhhn0r"] = np.ascontiguousarray(
        b_hh0[2 * H:].reshape(1, 2, P).astype(f16))
    shared["ones_rc"] = np.ones((1, RC), f16)
    shared["bhhn1"] = np.ascontiguousarray(
        b_hh1[2 * H:].reshape(2, P).T.astype(f32))
    shared["bhhn1r"] = np.ascontiguousarray(
        b_hh1[2 * H:].reshape(1, 2, P).astype(f16))

    def head_w(W):  # [NH, H, HD] -> [P, 2, NH, HD]
        return np.ascontiguousarray(
            W.transpose(1, 0, 2).reshape(2, P, NH, HD)
            .transpose(1, 0, 2, 3).astype(f16))

    shared["wq"] = head_w(d["Wq"])
    shared["wk"] = head_w(d["Wk"])
    shared["wv"] = head_w(d["Wv"])
    shared["bq"] = np.ascontiguousarray(d["bq"].T.astype(f32))  # [HD, NH]
    shared["bk"] = np.ascontiguousarray(d["bk"].T.astype(f32))
    shared["bv"] = np.ascontiguousarray(d["bv"].T.astype(f32))
    shared["bv16"] = np.ascontiguousarray(
        d["bv"].reshape(1, NH, HD).astype(f16))
    shared["rt"] = np.ascontiguousarray(
        d["R"].T.reshape(2, P, C).transpose(1, 0, 2).astype(f16))
    shared["wg"] = np.ascontiguousarray(
        d["W_gate"].reshape(2, P).T.reshape(P, 2, 1).astype(f16))
    shared["bg"] = np.asarray(d["b_gate"], f32).reshape(1, 1)
    shared["w1t"] = np.ascontiguousarray(
        d["W1"].T.reshape(2, P, 2, P).transpose(1, 0, 2, 3).astype(f16))
    shared["b1"] = np.ascontiguousarray(d["b1"].reshape(2, P).T.astype(f32))
    shared["w2"] = np.ascontiguousarray(
        d["W2"].reshape(2, P).T.reshape(P, 2, 1).astype(f16))
    shared["b2"] = np.asarray(d["b2"], f32).reshape(1, 1)
    shared["ident"] = np.eye(P, dtype=f16)
    shared["ones1"] = np.ones((1, P), f16)

    x = d["x"].astype(f32)
    in_maps = []
    for c in range(NCORES):
        xc = x[c * RC:(c + 1) * RC, :T]       # [RC, T, IN]
        xe = np.empty((T, KIN, RC), f16)
        xe[:, :IN, :] = xc.transpose(1, 2, 0).astype(f16)
        xe[:, IN, :] = 1.0
        m = dict(shared)
        m["xe"] = np.ascontiguousarray(xe)
        in_maps.append(m)
    return in_maps


def kernel(**inputs) -> np.ndarray:
    if "nc" not in _CACHE:
        _CACHE["nc"] = _build()
    nc = _CACHE["nc"]
    in_maps = _prep_host(inputs)
    trace = os.environ.get("BASS_KERNEL_TRACE", "0") == "1"
    try:
        res = run_bass_kernel_spmd(
            nc, in_maps, core_ids=list(range(NCORES)), trace=trace)
    except ModuleNotFoundError:
        # axon NTFF profile hook not present in this container; retry
        # with tracing disabled.
        os.environ["BASS_NEVER_TRACE"] = "1"
        res = run_bass_kernel_spmd(
            nc, in_maps, core_ids=list(range(NCORES)), trace=False)
    _CACHE["last_results"] = res
    y = np.asarray(res.results[0]["y_out"]).reshape(-1)[:N]
    return y.astype(np.float32)

